# revision 29
# baseline (speedup 1.0000x reference)
"""AdaGCL denoising 2-layer GCN on 8 Trainium2 NeuronCores (Bass/Tile).

Strategy (edge/graph parallelism, row-range sharded):
  - Host sorts edges by destination row; core k owns rows [k*18750, (k+1)*18750).
  - Per layer: dense per-node attention scalars a1/a2 (PE matmuls),
    all-gather a2; streaming pass A computes per-edge gates (mask) and a
    column-local cumulative sum of mask (triangular-matmul prefix sums);
    per-node degree = cumsum boundary differences (host-precomputed static
    boundary indices, indirect DMA gather).  d = clip(rowsum^-.5,0,10).
  - Pass B gathers x[col] rows (bf16, indirect DMA), forms msg = w * x[col],
    prefix-sums msg the same way, writes to DRAM, gathers boundary rows,
    differences -> segment sums, scales by d[row] -> layer output.
  - Layer 1 gathers from an all-gathered T = d1*x1 table so d[col] rides the
    row gather for free.
Output = x0 + x1 + x2 (accumulated on device), per-core slice -> concat.
"""
import math
import time
from dataclasses import dataclass

import numpy as np

GAMMA = -0.45
ZETA = 1.05
EPS_NOISE = 1e-7
EPS_DEG = 1e-6
N_NODES = 150000
N_EDGES = 1000000
DIM = 64
NCORES = 8

LAST_EXEC_NS = None


@dataclass(frozen=True)
class Cfg:
    n_nodes: int = N_NODES
    nslice: int = N_NODES // NCORES   # real nodes per core
    jp: int = 147                     # padded nodes per partition
    nblk: int = 32                    # pass-A blocks (4096 edges each) per core
    dim: int = DIM

    @property
    def npad(self):
        return 128 * self.jp

    @property
    def nmt(self):
        return 2 * self.nblk          # megatiles (2048 edges)

    @property
    def pad_e(self):
        return 4096 * self.nblk


REAL_CFG = Cfg()


# ---------------------------------------------------------------------------
# numpy fallback (also the host reference for self-checks)
# ---------------------------------------------------------------------------
def _numpy_kernel(features, row, col, noise0, noise1,
                  nbW0, nbb0, selfW0, selfb0, attW0, attb0,
                  nbW1, nbb1, selfW1, selfb1, attW1, attb1):
    x0 = np.asarray(features, np.float32)
    r = np.asarray(row).astype(np.int64)
    c = np.asarray(col).astype(np.int64)
    n_nodes = x0.shape[0]
    order = np.argsort(r, kind="stable")
    row_s = r[order]
    col_s = c[order]
    starts = np.flatnonzero(np.r_[True, row_s[1:] != row_s[:-1]])
    uniq = row_s[starts]
    n0 = np.asarray(noise0, np.float32)[:, 0][order]
    n1 = np.asarray(noise1, np.float32)[:, 0][order]

    def _layer(x, noise_s, nbW, nbb, selfW, selfb, attW, attb):
        d = x.shape[1]
        a1 = np.maximum(x @ nbW + nbb, 0.0) @ attW[:d, 0]
        a2 = np.maximum(x @ selfW + selfb, 0.0) @ attW[d:, 0]
        la = a1[row_s] + a2[col_s] + attb[0]
        u = np.clip(noise_s, EPS_NOISE, 1.0 - EPS_NOISE)
        gate = 1.0 / (1.0 + np.exp(-(np.log(u) - np.log1p(-u) + la)))
        mask = np.clip(gate * (ZETA - GAMMA) + GAMMA, 0.0, 1.0).astype(np.float32)
        rowsum = np.full(n_nodes, EPS_DEG, np.float32)
        rowsum[uniq] += np.add.reduceat(mask, starts)
        dd = np.clip(rowsum ** -0.5, 0.0, 10.0).astype(np.float32)
        vals = mask * dd[row_s] * dd[col_s]
        msg = vals[:, None] * x[col_s]
        out = np.zeros_like(x)
        out[uniq] = np.add.reduceat(msg, starts, axis=0)
        return out

    x1 = _layer(x0, n0, np.asarray(nbW0, np.float32), np.asarray(nbb0, np.float32),
                np.asarray(selfW0, np.float32), np.asarray(selfb0, np.float32),
                np.asarray(attW0, np.float32), np.asarray(attb0, np.float32))
    x2 = _layer(x1, n1, np.asarray(nbW1, np.float32), np.asarray(nbb1, np.float32),
                np.asarray(selfW1, np.float32), np.asarray(selfb1, np.float32),
                np.asarray(attW1, np.float32), np.asarray(attb1, np.float32))
    return (x0 + x1 + x2).astype(np.float32)


# ---------------------------------------------------------------------------
# Bass program
# ---------------------------------------------------------------------------

def _legalize_waits(nc):
    """This walrus build accepts only one semaphore wait per compute/DMA
    instruction: hoist extra waits onto standalone EventSemaphore
    instructions inserted just before, on the same engine."""
    from concourse import mybir
    multi_ok = ()
    k = 0
    for f in nc.m.functions:
        for b in f.blocks:
            out = []
            changed = False
            for ins in b.instructions:
                si = ins.sync_info
                if (si is not None and len(si.on_wait) > 1
                        and ins.engine is not None):
                    waits = list(si.on_wait)
                    for w in waits[:-1]:
                        ev = mybir.InstEventSemaphore(
                            name=f"LW-{k}", ins=[], outs=[])
                        k += 1
                        ev.engine = ins.engine
                        ev.sync_info = mybir.SyncInfo(on_wait=[w], on_update=[])
                        out.append(ev)
                    si.on_wait = [waits[-1]]
                    changed = True
                out.append(ins)
            if changed:
                b.instructions = out
    return k


def build_program(cfg: Cfg, layers=2, dbg=False, legalize=True):
    from concourse import bass, mybir
    import concourse.tile as tile
    from concourse.masks import make_identity, make_upper_triangular

    D = cfg.dim
    JP = cfg.jp
    NPAD = cfg.npad
    NSLICE = cfg.nslice
    NBLK = cfg.nblk
    NMT = cfg.nmt
    N = cfg.n_nodes
    f32 = mybir.dt.float32
    bf16 = mybir.dt.bfloat16
    i32 = mybir.dt.int32
    AG = [list(range(NCORES))]

    # output DMA split: partition p holds nodes [p*JP, (p+1)*JP)
    P0 = NSLICE // JP
    REM = NSLICE - P0 * JP

    nc = bass.Bass()

    # extra const APs used as activation biases
    for cval in (GAMMA,):
        _ct = nc.alloc_sbuf_tensor(f"const-f32-{cval}", [128, 1], f32)
        nc.gpsimd.memset(_ct.ap(), cval)
        nc.const_aps.aps[(f32, cval)] = _ct.ap()
    nc.all_engine_barrier()

    ein = {}

    def EIN(name, shape, dtype):
        ein[name] = (shape, dtype)
        return nc.dram_tensor(name, shape, dtype, kind="ExternalInput")

    x0r = EIN("x0r", [NPAD, D], f32)            # fp32 row slice (padded w/ 0)
    NSUB = NBLK * 32
    colg_d = EIN("colg", [NSUB, 128], i32)      # subtile-contiguous indices
    rowq_d = EIN("rowq", [NSUB, 128], i32)
    colq_d = EIN("colq", [NSUB, 128], i32)
    nz0_d = EIN("nz0", [NBLK, 128, 32], f32)
    nz1_d = EIN("nz1", [NBLK, 128, 32], f32)
    bs_d = EIN("bs", [JP + 1, 128], i32)        # scalar-cumsum boundaries (col-major)
    bzs_d = EIN("bzs", [JP, 128], i32)
    bv_d = EIN("bv", [JP + 1, 128], i32)        # vector-cumsum boundaries
    bzv_d = EIN("bzv", [JP, 128], i32)
    wdict = {}
    for nm in ["nbW0", "seW0", "nbW1", "seW1"]:
        wdict[nm] = EIN(nm, [D, D], f32)
    for nm in ["atA0", "atB0", "atA1", "atB1", "nbb0", "seb0", "nbb1", "seb1"]:
        wdict[nm] = EIN(nm, [D, 1], f32)
    for nm in ["attb0", "attb1"]:
        wdict[nm] = EIN(nm, [1, 1], f32)

    out_d = nc.dram_tensor("out", [NSLICE, D], f32, kind="ExternalOutput")

    a1s = nc.dram_tensor("a1s", [1, NPAD], f32)
    a2s = nc.dram_tensor("a2s", [1, NPAD], f32)
    a2f = nc.dram_tensor("a2f", [1, NCORES * NPAD], f32, addr_space="Shared")
    ds_ = nc.dram_tensor("ds", [1, NSLICE], f32)
    cumS = nc.dram_tensor("cumS", [NBLK + 1, 128, 32], f32)
    cumV = nc.dram_tensor("cumV", [NMT + 1, 128, 16, D], f32)
    x1b = nc.dram_tensor("x1b", [NSLICE, D], bf16)
    if dbg:
        dbg_mask = nc.dram_tensor("dbgmask", [NBLK, 128, 32], f32, kind="ExternalOutput")
        dbg_a1r = nc.dram_tensor("dbga1r", [NBLK, 128, 32], f32, kind="ExternalOutput")
        dbg_a2c = nc.dram_tensor("dbga2c", [NBLK, 128, 32], f32, kind="ExternalOutput")
        dbg_a1s = nc.dram_tensor("dbga1s", [1, NPAD], f32, kind="ExternalOutput")
        dbg_a2s = nc.dram_tensor("dbga2s", [1, NPAD], f32, kind="ExternalOutput")
        dbg_ds = nc.dram_tensor("dbgds", [1, NSLICE], f32, kind="ExternalOutput")
        dbg_cums = nc.dram_tensor("dbgcums", [NBLK + 1, 128, 32], f32, kind="ExternalOutput")
    x1tab = nc.dram_tensor("x1tab", [NCORES * NSLICE, D], bf16, addr_space="Shared")

    IOOA = bass.IndirectOffsetOnAxis

    with tile.TileContext(nc) as tc:
        with (
            tc.tile_pool(name="res", bufs=1) as res,
            tc.tile_pool(name="st", bufs=3) as st,
            tc.tile_pool(name="bd", bufs=2) as bd,
            tc.tile_pool(name="one", bufs=1) as one,
            tc.tile_pool(name="ps", bufs=2, space="PSUM") as ps,
            tc.tile_pool(name="pv", bufs=2, space="PSUM") as pvp,
        ):
            # ---- constants / resident state ----
            utf = res.tile([128, 128], f32)
            make_upper_triangular(nc, utf[:], val=1.0, diag=True)
            utb = res.tile([128, 128], bf16)
            nc.vector.tensor_copy(utb[:], utf[:])
            ident = res.tile([128, 128], f32)
            make_identity(nc, ident[:])

            acc = res.tile([128, JP, D], f32)       # x0 + x1 + x2 rows
            nc.sync.dma_start(acc[:], x0r[:].rearrange("(p j) d -> p j d", j=JP))
            xrow = res.tile([128, JP, D], f32)      # layer input rows (x0=acc)
            mask_r = res.tile([128, NBLK, 32], f32)
            dsl = res.tile([128, JP], f32)
            wtiles = {}
            for nm in ["nbW0", "seW0", "nbW1", "seW1"]:
                wtiles[nm] = res.tile([D, D], f32, tag=nm, name="w_" + nm)
                nc.sync.dma_start(wtiles[nm][:], wdict[nm][:])
            for nm in ["atA0", "atB0", "atA1", "atB1",
                       "nbb0", "seb0", "nbb1", "seb1"]:
                wtiles[nm] = res.tile([D, 1], f32, tag=nm, name="w_" + nm)
                nc.sync.dma_start(wtiles[nm][:], wdict[nm][:])
            for nm in ["attb0", "attb1"]:
                wtiles[nm] = res.tile([1, 1], f32, tag=nm, name="w_" + nm)
                nc.sync.dma_start(wtiles[nm][:], wdict[nm][:])

            # zero slots of the cumsum streams
            zv = one.tile([128, 16, D], f32, tag="zv")
            nc.vector.memset(zv[:], 0.0)
            nc.sync.dma_start(cumV[0], zv[:])
            zs = one.tile([128, 32], f32, tag="zs")
            nc.vector.memset(zs[:], 0.0)
            nc.sync.dma_start(cumS[0], zs[:])

            tc.strict_bb_all_engine_barrier()

            NG = (NPAD + 511) // 512  # dense node groups

            for L in range(layers):
                lay = str(L)
                src_rows = acc if L == 0 else xrow
                nbW = wtiles["nbW" + lay]
                seW = wtiles["seW" + lay]
                atA = wtiles["atA" + lay]
                atB = wtiles["atB" + lay]
                nbb = wtiles["nbb" + lay]
                seb = wtiles["seb" + lay]
                attb = wtiles["attb" + lay]
                nz_d = nz0_d if L == 0 else nz1_d

                # ===== dense phase: a1 (+attb), a2 per-node tables =====
                for g in range(NG):
                    j0 = g * 4               # node tile index within slice
                    w = min(512, NPAD - g * 512)
                    nt = w // 128
                    xT = st.tile([D, 512], f32, tag="xT")
                    for k in range(nt):
                        tp = ps.tile([D, 128], f32, tag="att")
                        nc.tensor.transpose(
                            tp[:], src_rows[:, j0 + k, :], ident[:])
                        nc.vector.tensor_copy(xT[:, k * 128:(k + 1) * 128], tp[:])
                    h1p = ps.tile([D, 512], f32, tag="h")
                    nc.tensor.matmul(h1p[:, :w], nbW[:], xT[:, :w])
                    h1 = st.tile([D, 512], f32, tag="h1")
                    nc.scalar.activation(
                        h1[:, :w], h1p[:, :w],
                        mybir.ActivationFunctionType.Relu, bias=nbb[:])
                    h2p = ps.tile([D, 512], f32, tag="h")
                    nc.tensor.matmul(h2p[:, :w], seW[:], xT[:, :w])
                    h2 = st.tile([D, 512], f32, tag="h2")
                    nc.scalar.activation(
                        h2[:, :w], h2p[:, :w],
                        mybir.ActivationFunctionType.Relu, bias=seb[:])
                    a1p = ps.tile([1, 512], f32, tag="att")
                    nc.tensor.matmul(a1p[:, :w], atA[:], h1[:, :w])
                    a2p = ps.tile([1, 512], f32, tag="att")
                    nc.tensor.matmul(a2p[:, :w], atB[:], h2[:, :w])
                    a1sb = st.tile([1, 512], f32, tag="a1sb")
                    nc.scalar.activation(
                        a1sb[:, :w], a1p[:, :w],
                        mybir.ActivationFunctionType.Identity, bias=attb[:])
                    a2sb = st.tile([1, 512], f32, tag="a2sb")
                    nc.vector.tensor_copy(a2sb[:, :w], a2p[:, :w])
                    nc.sync.dma_start(a1s[:, g * 512:g * 512 + w], a1sb[:, :w])
                    nc.sync.dma_start(a2s[:, g * 512:g * 512 + w], a2sb[:, :w])

                tc.strict_bb_all_engine_barrier()
                nc.gpsimd.collective_compute(
                    "AllGather", mybir.AluOpType.bypass, replica_groups=AG,
                    ins=[a2s[:]], outs=[a2f[:]])

                # ===== pass A: mask + scalar prefix sums =====
                for b in range(NBLK):
                    nz = st.tile([128, 32], f32, tag="nz")
                    nc.sync.dma_start(nz[:], nz_d[b])
                    a1r = st.tile([128, 32], f32, tag="a1r", bufs=4)
                    a2c = st.tile([128, 32], f32, tag="a2c", bufs=4)
                    for t in range(32):
                        s = b * 32 + t
                        rq_t = st.tile([128, 1], i32, tag="rq_t", bufs=16)
                        nc.sync.dma_start(rq_t[:], rowq_d[s].unsqueeze(-1))
                        cq_t = st.tile([128, 1], i32, tag="cq_t", bufs=16)
                        nc.scalar.dma_start(cq_t[:], colq_d[s].unsqueeze(-1))
                        a1t = st.tile([128, 1], f32, tag="a1t", bufs=32)
                        nc.gpsimd.indirect_dma_start(
                            out=a1t[:], out_offset=None,
                            in_=a1s[:], in_offset=IOOA(ap=rq_t[:], axis=1))
                        nc.vector.tensor_copy(a1r[:, t:t + 1], a1t[:])
                        a2t = st.tile([128, 1], f32, tag="a2t", bufs=32)
                        nc.gpsimd.indirect_dma_start(
                            out=a2t[:], out_offset=None,
                            in_=a2f[:], in_offset=IOOA(ap=cq_t[:], axis=1))
                        nc.scalar.activation(
                            a2c[:, t:t + 1], a2t[:],
                            mybir.ActivationFunctionType.Identity)
                    nc.vector.tensor_scalar_max(nz[:], nz[:], EPS_NOISE)
                    nc.vector.tensor_scalar_min(nz[:], nz[:], 1.0 - EPS_NOISE)
                    lgu = st.tile([128, 32], f32, tag="lgu")
                    nc.scalar.activation(lgu[:], nz[:],
                                         mybir.ActivationFunctionType.Ln)
                    lg1m = st.tile([128, 32], f32, tag="lg1m")
                    nc.scalar.activation(lg1m[:], nz[:],
                                         mybir.ActivationFunctionType.Ln,
                                         scale=-1.0, bias=1.0)
                    nc.vector.tensor_sub(lgu[:], lgu[:], lg1m[:])
                    nc.vector.tensor_add(a1r[:], a1r[:], a2c[:])
                    nc.vector.tensor_add(lgu[:], lgu[:], a1r[:])
                    gt = st.tile([128, 32], f32, tag="gt")
                    nc.scalar.activation(gt[:], lgu[:],
                                         mybir.ActivationFunctionType.Sigmoid)
                    nc.scalar.activation(gt[:], gt[:],
                                         mybir.ActivationFunctionType.Relu,
                                         scale=(ZETA - GAMMA), bias=GAMMA)
                    nc.vector.tensor_scalar_min(mask_r[:, b, :], gt[:], 1.0)
                    if dbg and L == 0:
                        nc.sync.dma_start(dbg_mask[b], mask_r[:, b, :])
                        nc.sync.dma_start(dbg_a1r[b], a1r[:])
                        nc.sync.dma_start(dbg_a2c[b], a2c[:])
                    csp = ps.tile([128, 32], f32, tag="att")
                    nc.tensor.matmul(csp[:], utf[:], mask_r[:, b, :])
                    css = st.tile([128, 32], f32, tag="css")
                    nc.vector.tensor_copy(css[:], csp[:])
                    nc.sync.dma_start(cumS[b + 1], css[:])

                if dbg and L == 0:
                    nc.sync.dma_start(dbg_a1s[:], a1s[:])
                    nc.sync.dma_start(dbg_a2s[:], a2s[:])
                    nc.sync.dma_start(dbg_cums[:], cumS[:])
                tc.strict_bb_all_engine_barrier()
                # rowsum via boundary differences -> d
                gb = st.tile([128, JP + 1], f32, tag="gb")
                for j in range(JP + 1):
                    bs_j = st.tile([128, 1], i32, tag="bs_j", bufs=16)
                    nc.sync.dma_start(bs_j[:], bs_d[j].unsqueeze(-1))
                    nc.gpsimd.indirect_dma_start(
                        out=gb[:, j:j + 1], out_offset=None,
                        in_=cumS[:], in_offset=IOOA(ap=bs_j[:], axis=2))
                gz = st.tile([128, JP], f32, tag="gz")
                for j in range(JP):
                    bz_j = st.tile([128, 1], i32, tag="bz_j", bufs=16)
                    nc.scalar.dma_start(bz_j[:], bzs_d[j].unsqueeze(-1))
                    nc.gpsimd.indirect_dma_start(
                        out=gz[:, j:j + 1], out_offset=None,
                        in_=cumS[:], in_offset=IOOA(ap=bz_j[:], axis=2))
                rs = st.tile([128, JP], f32, tag="rs")
                nc.vector.tensor_sub(rs[:], gb[:, 1:], gb[:, :JP])
                nc.vector.tensor_add(rs[:], rs[:], gz[:])
                nc.vector.tensor_scalar_add(rs[:], rs[:], EPS_DEG)
                nc.vector.reciprocal(rs[:], rs[:])
                nc.scalar.activation(dsl[:], rs[:],
                                     mybir.ActivationFunctionType.Sqrt)
                nc.vector.tensor_scalar_min(dsl[:], dsl[:], 10.0)

                if dbg and L == 0:
                    nc.sync.dma_start(
                        ds_[0, :P0 * JP].rearrange("(p j) -> p j", j=JP),
                        dsl[:P0, :])
                    nc.sync.dma_start(dbg_ds[:], ds_[:])
                # T = d * x rows -> bf16 -> allgather gather table
                t1 = one.tile([128, JP, D], bf16, tag="t1")
                nc.vector.tensor_tensor(
                    out=t1[:], in0=src_rows[:],
                    in1=dsl[:].to_broadcast([128, JP, D]),
                    op=mybir.AluOpType.mult)
                nc.sync.dma_start(
                    x1b[:P0 * JP, :].rearrange("(p j) d -> p j d", j=JP),
                    t1[:P0, :, :])
                if REM:
                    nc.sync.dma_start(
                        x1b[P0 * JP:, :].rearrange("(o j) d -> o j d", o=1),
                        t1[P0:P0 + 1, :REM, :])
                nc.gpsimd.collective_compute(
                    "AllGather", mybir.AluOpType.bypass, replica_groups=AG,
                    ins=[x1b[:]], outs=[x1tab[:]])
                gtab = x1tab

                tc.strict_bb_all_engine_barrier()
                # ===== pass B: gather + msg + vector prefix sums =====
                for m in range(NMT):
                    b, half = divmod(m, 2)
                    mkv = mask_r[:, b, half * 16:half * 16 + 16]
                    xc = st.tile([128, 16, D], bf16, tag="xc", bufs=4)
                    for t in range(16):
                        s = m * 16 + t
                        cg_t = st.tile([128, 1], i32, tag="cg_t", bufs=16)
                        nc.sync.dma_start(cg_t[:], colg_d[s].unsqueeze(-1))
                        xct = st.tile([128, D], bf16, tag="xct", bufs=16)
                        nc.gpsimd.indirect_dma_start(
                            out=xct[:], out_offset=None,
                            in_=gtab[:], in_offset=IOOA(ap=cg_t[:], axis=0))
                        if t % 2 == 0:
                            nc.vector.tensor_copy(xc[:, t, :], xct[:])
                        else:
                            nc.scalar.activation(
                                xc[:, t, :], xct[:],
                                mybir.ActivationFunctionType.Identity)
                    w16 = st.tile([128, 16], bf16, tag="w16")
                    nc.vector.tensor_copy(w16[:], mkv)
                    msg = st.tile([128, 16, D], bf16, tag="msg")
                    nc.vector.tensor_tensor(
                        out=msg[:], in0=xc[:],
                        in1=w16[:].to_broadcast([128, 16, D]),
                        op=mybir.AluOpType.mult)
                    pv = pvp.tile([128, 16, D], f32, tag="pv")
                    nc.tensor.matmul(
                        pv[:, :8, :].rearrange("p a b -> p (a b)"), utb[:],
                        msg[:, :8, :].rearrange("p a b -> p (a b)"))
                    nc.tensor.matmul(
                        pv[:, 8:, :].rearrange("p a b -> p (a b)"), utb[:],
                        msg[:, 8:, :].rearrange("p a b -> p (a b)"))
                    cv = st.tile([128, 16, D], f32, tag="cv")
                    if m % 2 == 0:
                        nc.vector.tensor_copy(cv[:], pv[:])
                    else:
                        nc.scalar.activation(
                            cv[:], pv[:], mybir.ActivationFunctionType.Identity)
                    nc.sync.dma_start(cumV[m + 1], cv[:])

                tc.strict_bb_all_engine_barrier()
                # ===== boundary differences -> layer output rows =====
                xout = xrow  # L0: x1 rows; L1: x2 rows (scratch reuse)
                nch = 6
                csz = (JP + nch - 1) // nch
                for h in range(nch):
                    c0 = h * csz
                    cw = min(csz, JP - c0)
                    if cw <= 0:
                        continue
                    GB = bd.tile([128, csz + 1, D], f32, tag="GB")
                    for j in range(cw + 1):
                        bv_j = st.tile([128, 1], i32, tag="bv_j", bufs=16)
                        nc.sync.dma_start(bv_j[:], bv_d[c0 + j].unsqueeze(-1))
                        nc.gpsimd.indirect_dma_start(
                            out=GB[:, j, :], out_offset=None,
                            in_=cumV[:].rearrange("s p t d -> (s p t) d"),
                            in_offset=IOOA(ap=bv_j[:], axis=0))
                    GZ = bd.tile([128, csz, D], f32, tag="GZ")
                    for j in range(cw):
                        bzv_j = st.tile([128, 1], i32, tag="bzv_j", bufs=16)
                        nc.scalar.dma_start(bzv_j[:], bzv_d[c0 + j].unsqueeze(-1))
                        nc.gpsimd.indirect_dma_start(
                            out=GZ[:, j, :], out_offset=None,
                            in_=cumV[:].rearrange("s p t d -> (s p t) d"),
                            in_offset=IOOA(ap=bzv_j[:], axis=0))
                    sl = xout[:, c0:c0 + cw, :]
                    nc.vector.tensor_sub(sl, GB[:, 1:cw + 1, :], GB[:, :cw, :])
                    nc.vector.tensor_add(sl, sl, GZ[:, :cw, :])
                    nc.vector.tensor_tensor(
                        out=sl, in0=sl,
                        in1=dsl[:, c0:c0 + cw].to_broadcast([128, cw, D]),
                        op=mybir.AluOpType.mult)
                    nc.vector.tensor_add(
                        acc[:, c0:c0 + cw, :], acc[:, c0:c0 + cw, :], sl)

            # ===== final output =====
            nc.sync.dma_start(
                out_d[:P0 * JP, :].rearrange("(p j) d -> p j d", j=JP),
                acc[:P0, :, :])
            if REM:
                nc.sync.dma_start(out_d[P0 * JP:, :].rearrange("(o j) d -> o j d", o=1),
                          acc[P0:P0 + 1, :REM, :])

    if legalize:
        _legalize_waits(nc)
    return nc, ein


# ---------------------------------------------------------------------------
# host preprocessing
# ---------------------------------------------------------------------------
def host_prep(cfg: Cfg, features, row, col, noise0, noise1,
              nbW0, nbb0, selfW0, selfb0, attW0, attb0,
              nbW1, nbb1, selfW1, selfb1, attW1, attb1):
    import ml_dtypes

    D = cfg.dim
    JP = cfg.jp
    NPAD = cfg.npad
    NSLICE = cfg.nslice
    NBLK = cfg.nblk
    NMT = cfg.nmt
    PAD_E = cfg.pad_e
    N = cfg.n_nodes

    x0 = np.ascontiguousarray(np.asarray(features, np.float32))
    r = np.asarray(row).astype(np.int64)
    c = np.asarray(col).astype(np.int64)
    order = np.argsort(r, kind="stable")
    rs = r[order]
    cs = c[order]
    n0 = np.asarray(noise0, np.float32).reshape(-1)[order]
    n1 = np.asarray(noise1, np.float32).reshape(-1)[order]


    def vrow(s_arr):
        m = s_arr >> 11
        p = s_arr & 127
        t = (s_arr >> 7) & 15
        out = (m + 1) * 2048 + p * 16 + t
        return np.where(s_arr < 0, 0, out).astype(np.int32)

    def srow(s_arr):
        b = s_arr >> 12
        p = s_arr & 127
        cb = (s_arr >> 7) & 31
        out = (b + 1) * 4096 + p * 32 + cb
        return np.where(s_arr < 0, 0, out).astype(np.int32)

    bounds = np.searchsorted(rs, NSLICE * np.arange(NCORES + 1))
    in_maps = []
    common = dict(
        nbW0=np.asarray(nbW0, np.float32), seW0=np.asarray(selfW0, np.float32),
        nbW1=np.asarray(nbW1, np.float32), seW1=np.asarray(selfW1, np.float32),
        atA0=np.ascontiguousarray(np.asarray(attW0, np.float32)[:D]),
        atB0=np.ascontiguousarray(np.asarray(attW0, np.float32)[D:]),
        atA1=np.ascontiguousarray(np.asarray(attW1, np.float32)[:D]),
        atB1=np.ascontiguousarray(np.asarray(attW1, np.float32)[D:]),
        nbb0=np.asarray(nbb0, np.float32).reshape(D, 1),
        seb0=np.asarray(selfb0, np.float32).reshape(D, 1),
        nbb1=np.asarray(nbb1, np.float32).reshape(D, 1),
        seb1=np.asarray(selfb1, np.float32).reshape(D, 1),
        attb0=np.asarray(attb0, np.float32).reshape(1, 1),
        attb1=np.asarray(attb1, np.float32).reshape(1, 1),
    )

    for k in range(NCORES):
        lo, hi = bounds[k], bounds[k + 1]
        ec = hi - lo
        if ec > PAD_E:
            raise RuntimeError(f"core {k} edge count {ec} > PAD_E {PAD_E}")

        def padded(a, fill, dtype):
            outa = np.full(PAD_E, fill, dtype)
            outa[:ec] = a
            return outa

        def qmap(n):
            return (n % JP) * 128 + n // JP

        rl = padded(qmap(rs[lo:hi] - k * NSLICE), qmap(NSLICE - 1), np.int32)
        cg = padded(cs[lo:hi], 0, np.int32)
        cq = padded((cs[lo:hi] // NSLICE) * NPAD + qmap(cs[lo:hi] % NSLICE),
                    0, np.int32)
        z0 = padded(n0[lo:hi], 0.0, np.float32)
        z1 = padded(n1[lo:hi], 0.0, np.float32)

        def lay(a):
            # stream s = b*4096 + cb*128 + p  ->  [b, p, cb]
            return np.ascontiguousarray(
                a.reshape(NBLK, 32, 128).transpose(0, 2, 1))

        # boundary stream positions per node (real edges only)
        cnt = np.searchsorted(rs[lo:hi], np.arange(NSLICE) + k * NSLICE,
                              side="right")
        s1 = np.full(NPAD, ec - 1, np.int64)
        s1[:NSLICE] = cnt - 1          # -1 if no edges yet
        s1[NSLICE:] = ec - 1
        s0 = np.empty(NPAD, np.int64)
        s0[0] = -1
        s0[1:] = s1[:-1]

        col1 = np.where(s1 >= 0, s1 >> 7, -1)
        col0 = np.where(s0 >= 0, s0 >> 7, -1)
        spans = (s1 > s0) & (col1 > np.maximum(col0, 0)) | \
                ((s0 < 0) & (col1 > 0))
        if np.any(col1 - np.maximum(col0, 0) > 1):
            raise RuntimeError("node degree spans >2 cumsum columns")
        zpos = np.where(spans,
                        (np.maximum(col0, 0) + 1) * 128 - 1,
                        -1)

        # node n -> (p = n // JP, j = n % JP)
        e_nodes = np.full((128, JP + 1), -1, np.int64)
        z_nodes = np.full((128, JP), -1, np.int64)
        s1m = s1.reshape(128, JP)
        s0m = s0.reshape(128, JP)
        zm = zpos.reshape(128, JP)
        e_nodes[:, 0] = s0m[:, 0]
        e_nodes[:, 1:] = s1m
        z_nodes[:, :] = zm

        in_map = dict(common)
        in_map.update(
            x0r=np.concatenate(
                [x0[k * NSLICE:(k + 1) * NSLICE],
                 np.zeros((NPAD - NSLICE, D), np.float32)], axis=0),
            colg=cg.reshape(-1, 128), rowq=rl.reshape(-1, 128),
            colq=cq.reshape(-1, 128),
            nz0=lay(z0), nz1=lay(z1),
            bs=np.ascontiguousarray(srow(e_nodes).T),
            bzs=np.ascontiguousarray(srow(z_nodes).T),
            bv=np.ascontiguousarray(vrow(e_nodes).T),
            bzv=np.ascontiguousarray(vrow(z_nodes).T),
        )
        in_maps.append(in_map)
    return in_maps


# ---------------------------------------------------------------------------
# PJRT runner (axon): build jit once, reuse for result + timing
# ---------------------------------------------------------------------------
_RUNNER_CACHE = {}


def _get_runner(cfg: Cfg):
    key = cfg
    if key in _RUNNER_CACHE:
        return _RUNNER_CACHE[key]
    import jax
    from jax.sharding import Mesh, PartitionSpec, NamedSharding
    from jax.experimental.shard_map import shard_map
    from concourse import bass2jax, mybir

    nc, ein = build_program(cfg)
    bass2jax.install_neuronx_cc_hook()

    in_names, out_names, out_avals, zero_shapes = [], [], [], []
    partition_name = nc.partition_id_tensor.name if nc.partition_id_tensor else None
    for alloc in nc.m.functions[0].allocations:
        if not isinstance(alloc, mybir.MemoryLocationSet):
            continue
        name = alloc.memorylocations[0].name
        if alloc.kind == "ExternalInput":
            if name != partition_name:
                in_names.append(name)
        elif alloc.kind == "ExternalOutput":
            out_names.append(name)
            shape = tuple(alloc.tensor_shape)
            dtype = mybir.dt.np(alloc.dtype)
            out_avals.append(jax.core.ShapedArray(shape, dtype))
            zero_shapes.append((shape, dtype))
    n_params = len(in_names)
    all_names = list(in_names) + list(out_names)
    if partition_name is not None:
        all_names.append(partition_name)
    donate = tuple(range(n_params, n_params + len(out_names)))

    def _body(*args):
        operands = list(args)
        if partition_name is not None:
            operands.append(bass2jax.partition_id_tensor())
        outs = bass2jax._bass_exec_p.bind(
            *operands,
            out_avals=tuple(out_avals),
            in_names=tuple(all_names),
            out_names=tuple(out_names),
            lowering_input_output_aliases=(),
            sim_require_finite=True,
            sim_require_nnan=True,
            nc=nc,
        )
        return tuple(outs)

    devices = jax.devices()[:NCORES]
    mesh = Mesh(np.asarray(devices), ("core",))
    in_specs = (PartitionSpec("core"),) * (n_params + len(out_names))
    out_specs = (PartitionSpec("core"),) * len(out_names)
    sharded = jax.jit(
        shard_map(_body, mesh=mesh, in_specs=in_specs, out_specs=out_specs,
                  check_rep=False),
        donate_argnums=donate, keep_unused=True)

    runner = dict(nc=nc, sharded=sharded, in_names=in_names,
                  out_names=out_names, zero_shapes=zero_shapes, mesh=mesh,
                  sharding=NamedSharding(mesh, PartitionSpec("core")))
    _RUNNER_CACHE[key] = runner
    return runner


def run_on_device(cfg: Cfg, in_maps, time_it=True, n_timing=3):
    import jax

    r = _get_runner(cfg)
    sharded = r["sharded"]
    shd = r["sharding"]

    concat_in = [
        jax.device_put(
            np.concatenate([np.asarray(m[name]) for m in in_maps], axis=0), shd)
        for name in r["in_names"]
    ]

    def zeros():
        return [
            jax.device_put(
                np.zeros((NCORES * s[0],) + tuple(s[1:]), dt), shd)
            for s, dt in r["zero_shapes"]
        ]

    outs = sharded(*concat_in, *zeros())
    outs = [np.asarray(o) for o in jax.block_until_ready(outs)]

    exec_ns = None
    if time_it:
        times = []
        for _ in range(n_timing):
            z = zeros()
            jax.block_until_ready(z)
            jax.block_until_ready(concat_in)
            t0 = time.perf_counter()
            o = sharded(*concat_in, *z)
            jax.block_until_ready(o)
            times.append(time.perf_counter() - t0)
        exec_ns = int(min(times) * 1e9)

    results = []
    for i, name in enumerate(r["out_names"]):
        per_core = outs[i].reshape(NCORES, -1, *outs[i].shape[1:])
        results.append((name, per_core))
    return dict(results), exec_ns


# ---------------------------------------------------------------------------
# entry point
# ---------------------------------------------------------------------------
def kernel(features, row, col, noise0, noise1,
           nbW0, nbb0, selfW0, selfb0, attW0, attb0,
           nbW1, nbb1, selfW1, selfb1, attW1, attb1):
    global LAST_EXEC_NS
    cfg = REAL_CFG
    args = (features, row, col, noise0, noise1,
            nbW0, nbb0, selfW0, selfb0, attW0, attb0,
            nbW1, nbb1, selfW1, selfb1, attW1, attb1)
    try:
        in_maps = host_prep(cfg, *args)
    except RuntimeError as e:
        print(f"kernel: host prep failed ({e}); numpy fallback")
        return _numpy_kernel(*args)
    outs, exec_ns = run_on_device(cfg, in_maps)
    LAST_EXEC_NS = exec_ns
    per_core = outs["out"]  # [NCORES, NSLICE, D]
    return np.ascontiguousarray(
        per_core.reshape(NCORES * cfg.nslice, cfg.dim)).astype(np.float32)
